# revision 13
# baseline (speedup 1.0000x reference)
"""Trainium2 Bass kernel for ColorProcessingLossV2 (8-core data-parallel).

Pipeline per core (one batch image, 3x512x1024 pixels laid out as [128, 4096]):
  phase 1: chanmax = max over RGB of org_img; v = chanmax - NIGHT[seg]
           (19-entry gather realized as an 18-step staircase of custom
           fused DVE ops, 2 steps per op); local min/max of v.
  sync:    AllGather of [vmax, -vmin] across the 8 cores; each core
           reduces to the global imin/imax.
  phase 2: g = ((v - imin)/(imax - imin))^0.4 computed as
           exp(0.4*ln(v - imin) - 0.4*ln(imax - imin)) on the ACT engine;
           w = g + (CITY[seg] - CITY[0]) via the second staircase;
           per-channel partial sums of min(w + CITY[0], 1) * reflectance
           via a fused clip-mul-reduce DVE op.
  output:  per-core [128, 3] per-partition channel sums; host reduces in
           float64 and forms the scalar loss.
"""

import os
import sys

for _p in ("/root/.axon_site/_ro/trn_rl_repo", "/opt/trn_rl_repo"):
    if _p not in sys.path:
        sys.path.append(_p)

import numpy as np


def _install_ntff_hook_shim():
    """Provide antenv.axon_hooks (NTFF profiling hook) when the image lacks it."""
    try:
        import antenv.axon_hooks  # noqa: F401
        return
    except ImportError:
        pass
    import contextlib
    import ctypes
    import types

    so_path = "/opt/axon/libaxon_pjrt.so"
    state = {"hook": None}

    def _make_hook():
        if not os.path.exists(so_path):
            return None
        lib = ctypes.CDLL(so_path)
        if not hasattr(lib, "axon_start_nrt_profile"):
            return None
        lib.axon_start_nrt_profile.argtypes = [
            ctypes.POINTER(ctypes.c_int64), ctypes.c_size_t]
        lib.axon_start_nrt_profile.restype = ctypes.c_int64
        lib.axon_stop_nrt_profile.argtypes = [ctypes.c_char_p]
        lib.axon_stop_nrt_profile.restype = ctypes.c_int64

        @contextlib.contextmanager
        def _hook(output_dir, device_ids):
            import jax
            jax.devices()
            if device_ids:
                ids = (ctypes.c_int64 * len(device_ids))(*device_ids)
                rc = lib.axon_start_nrt_profile(ids, len(device_ids))
            else:
                rc = lib.axon_start_nrt_profile(None, 0)
            if rc != 0:
                raise RuntimeError(f"axon_start_nrt_profile rc={rc}")
            try:
                yield
            finally:
                n = lib.axon_stop_nrt_profile(str(output_dir).encode())
                print(f"profile: {n} file(s) written to {output_dir}",
                      file=sys.stderr)

        return _hook

    mod = types.ModuleType("antenv.axon_hooks")

    def get_axon_ntff_profile_hook():
        if state["hook"] is None:
            state["hook"] = _make_hook()
        return state["hook"]

    def set_axon_ntff_profile_hook(h):
        state["hook"] = h

    mod.get_axon_ntff_profile_hook = get_axon_ntff_profile_hook
    mod.set_axon_ntff_profile_hook = set_axon_ntff_profile_hook
    sys.modules["antenv.axon_hooks"] = mod


_install_ntff_hook_shim()

import concourse.bacc as bacc
import concourse.bass_isa as bass_isa
import concourse.mybir as mybir
import concourse.tile as tile
import concourse.dve_ops as dve_ops
from concourse.bass_utils import run_bass_kernel_spmd
from concourse.dve_spec import Spec, Src0, Src1, C0, C1, C2, One, minn, maxx, lower, _has_src1
from concourse.dve_uop import DveOpSpec
from operator import add as _operator_add

# ---------------------------------------------------------------- constants

_NIGHT = np.array([
    76.5113984140019, 76.23163212875781, 60.90662084364415, 69.06930071129905,
    69.63671393061327, 73.11413822794262, 140.7827781957324, 116.29554873008291,
    46.23329954488532, 57.839322341112386, 32.61465346757989, 57.4385179294615,
    62.234896087294814, 90.90285758569436, 91.99610158117673, 91.82209397173472,
    94.06478985576457, 74.6924145472464, 69.15034088822232], dtype=np.float64) / 255.0
_CITY = np.array([
    86.46051320057052, 79.37014543897092, 95.30679177391578, 71.11888521745776,
    75.57026559270716, 77.90493757655786, 74.77466800282637, 88.27701037425895,
    57.685269557270146, 72.71472387765841, 229.9589238353863, 66.9194012998903,
    60.42471796718752, 76.8407421534007, 74.98657626719087, 73.56771430328095,
    123.92515568872523, 68.93476495876828, 76.0970460111028], dtype=np.float64) / 255.0

# staircase deltas over the fp32-rounded tables (matches the reference's
# fp32 table values at every integer label)
_DN = np.diff(_NIGHT.astype(np.float32).astype(np.float64))
_DC = np.diff(_CITY.astype(np.float32).astype(np.float64))
_C0 = float(np.float32(_CITY[0]))

GAMMA = 0.4
LOSS_WEIGHT = 1.0

B, C, H, W_IMG = 8, 3, 512, 1024
P = 128
F = (H * W_IMG) // P          # 4096 free elements per partition
N_CORES = 8
NCH = 4                        # chunks per phase
CW = F // NCH                  # chunk width

# ---------------------------------------------------------------- custom ops


def _register_dve_op(name, spec, subdim=False):
    for op in dve_ops.OPS:
        if op.name == name:
            return op
    row = dve_ops._CUSTOM_DVE_ROW_BASE + len(dve_ops.OPS)
    assert row < 0x20, "custom DVE opcode rows exhausted"
    shas = {}
    for ver in ("v3", "v4"):
        tmp = DveOpSpec(name=name, opcode=row, uops=lower(spec, ver=ver),
                        rd1_en=_has_src1(spec))
        shas[ver] = tmp.sha(ver)
    op = dve_ops.DveOp(name, spec, subdim=subdim, uops_sha=shas)
    dve_ops.OPS.append(op)
    dve_ops.CUSTOM_DVE_SPECS[name] = spec
    dve_ops._SUB_OPCODE_FOR_NAME[name] = row
    return op


def _stair2_ref(in0, in1, s0, s1, imm2):
    x = in0.astype(np.float32)
    return (in1 + (x >= s0) * np.float32(s1) + (x > s0) * np.float32(imm2)
            ).astype(np.float32)


# out = acc + (seg >= k) * d0 + (seg > k) * d1   (two staircase steps)
STAIR2 = _register_dve_op(
    "STAIR2_ANT",
    Spec(body=Src1 + (Src0 >= C0) * C1 + (Src0 > C0) * C2,
         reference=_stair2_ref),
)


def _stair2_amax_ref(in0, in1, s0, s1, imm2):
    body = _stair2_ref(in0, in1, s0, s1, imm2)
    acc = np.maximum.reduce(body.reshape(body.shape[0], -1), axis=-1,
                            keepdims=True)
    return body, acc.astype(np.float32)


# STAIR2 + running max of the output (used on the last NIGHT step)
STAIR2_AMAX = _register_dve_op(
    "STAIR2_AMAX_ANT",
    Spec(body=Src1 + (Src0 >= C0) * C1 + (Src0 > C0) * C2,
         accum=maxx, reference=_stair2_amax_ref),
)


def _stair2_init_ref(in0, in1, s0, s1, imm2):
    x = in0.astype(np.float32)
    return ((x >= s0) * np.float32(s1) + (x > s0) * np.float32(imm2)
            ).astype(np.float32)


# first staircase op of a zero-seeded chain (no accumulator input)
STAIR2_INIT = _register_dve_op(
    "STAIR2_INIT_ANT",
    Spec(body=(Src0 >= C0) * C1 + (Src0 > C0) * C2,
         reference=_stair2_init_ref),
)


def _cmr_ref(in0, in1, s0, s1, imm2):
    body = (np.minimum(in0.astype(np.float32) + np.float32(s0), np.float32(1.0))
            * in1).astype(np.float32)
    acc = (np.asarray(s1, np.float32).reshape(-1, 1)
           + body.reshape(body.shape[0], -1).sum(axis=-1, keepdims=True))
    return body, acc.astype(np.float32)


# out = min(w + c0, 1) * refl ; accum_out = seed + sum(out)
CLIPMULRED = _register_dve_op(
    "CLIPMULRED_ANT",
    Spec(body=minn(Src0 + C0, One) * Src1, accum=_operator_add,
         accum_init=C1, reference=_cmr_ref),
)

# ---------------------------------------------------------------- bass build

_COMPILED = None


def _build():
    dt = mybir.dt
    nc = bacc.Bacc("TRN2", target_bir_lowering=False, debug=False,
                   num_devices=N_CORES)
    org_d = nc.dram_tensor("org", [C, P, F], dt.float32, kind="ExternalInput").ap()
    seg_d = nc.dram_tensor("seg", [P, F], dt.int32, kind="ExternalInput").ap()
    refl_d = nc.dram_tensor("refl", [C, P, F], dt.float32, kind="ExternalInput").ap()
    out_d = nc.dram_tensor("out", [P, C], dt.float32, kind="ExternalOutput").ap()

    with tile.TileContext(nc) as tc:
        with (
            tc.tile_pool(name="pers", bufs=1) as pers,
            tc.tile_pool(name="work", bufs=2) as work,
            tc.tile_pool(name="dram", bufs=1, space="DRAM") as dram,
        ):
            seg = pers.tile([P, F], dt.int32, tag="seg")
            v = pers.tile([P, F], dt.float32, tag="v")
            cp = pers.tile([P, F], dt.float32, tag="cp")

            # ---- phase 1: chanmax, NIGHT staircase, local min/max ----
            mxs, mns = [], []
            for ch in range(NCH):
                sl = slice(ch * CW, (ch + 1) * CW)
                nc.sync.dma_start(seg[:, sl], seg_d[:, sl])
                o01 = work.tile([P, 2 * CW], dt.float32, tag="org01")
                nc.sync.dma_start(o01[:, 0:CW], org_d[0, :, sl])
                nc.sync.dma_start(o01[:, CW:2 * CW], org_d[1, :, sl])
                o2 = work.tile([P, CW], dt.float32, tag="org2")
                nc.sync.dma_start(o2[:], org_d[2, :, sl])
                cmx = work.tile([P, CW], dt.float32, tag="cmx")
                nc.vector.tensor_tensor(cmx[:], o01[:, 0:CW], o01[:, CW:2 * CW],
                                        mybir.AluOpType.max)
                nc.vector.tensor_tensor(cmx[:], cmx[:], o2[:],
                                        mybir.AluOpType.max)
                # NIGHT staircase: v = chanmax - (NIGHT[seg] - NIGHT[0])
                sa = work.tile([P, CW], dt.float32, tag="sa")
                cur = cmx[:]
                mx = pers.tile([P, 1], dt.float32, tag=f"mx{ch}")
                for j in range(9):
                    k = 1 + 2 * j
                    if j == 8:
                        nc.vector._custom_dve(
                            STAIR2_AMAX, out=v[:, sl], in0=seg[:, sl], in1=cur,
                            s0=float(k), s1=float(-_DN[k - 1]),
                            imm2=float(-_DN[k]), accum_out=mx[:])
                    else:
                        outt = sa[:] if j % 2 == 0 else cmx[:]
                        nc.vector._custom_dve(
                            STAIR2, out=outt, in0=seg[:, sl], in1=cur,
                            s0=float(k), s1=float(-_DN[k - 1]),
                            imm2=float(-_DN[k]))
                        cur = outt
                mn = pers.tile([P, 1], dt.float32, tag=f"mn{ch}")
                nc.vector.tensor_reduce(mn[:], v[:, sl], mybir.AxisListType.X,
                                        mybir.AluOpType.min)
                mxs.append(mx)
                mns.append(mn)

            # combine chunk extremes -> [128, 2] = [vmax, -vmin]
            mm = pers.tile([P, 2], dt.float32, tag="mm")
            nc.vector.tensor_tensor(mm[:, 0:1], mxs[0][:], mxs[1][:],
                                    mybir.AluOpType.max)
            for t in mxs[2:]:
                nc.vector.tensor_tensor(mm[:, 0:1], mm[:, 0:1], t[:],
                                        mybir.AluOpType.max)
            nc.vector.tensor_tensor(mm[:, 1:2], mns[0][:], mns[1][:],
                                    mybir.AluOpType.min)
            for t in mns[2:]:
                nc.vector.tensor_tensor(mm[:, 1:2], mm[:, 1:2], t[:],
                                        mybir.AluOpType.min)
            nc.vector.tensor_scalar(mm[:, 1:2], mm[:, 1:2], -1.0, None,
                                    mybir.AluOpType.mult)

            # AllGather the per-core per-partition [vmax, -vmin]
            # (overlaps the CITY stairs)
            agi = dram.tile([P, 2], dt.float32)
            ago = dram.tile([N_CORES * P, 2], dt.float32, addr_space="Shared")
            nc.gpsimd.dma_start(agi[:], mm[:])
            nc.gpsimd.collective_compute(
                "AllGather", mybir.AluOpType.bypass,
                replica_groups=[list(range(N_CORES))],
                ins=[agi.opt()], outs=[ago.opt()])
            agt = pers.tile([1, 2 * N_CORES * P], dt.float32, tag="agt")
            nc.sync.dma_start(agt[:], ago[:])
            # reduce the 1024 [vmax, -vmin] pairs on partition 0
            g1 = pers.tile([1, 2], dt.float32, tag="g1")
            nc.vector.tensor_reduce(
                g1[:], agt[:].rearrange("p (c two) -> p two c", two=2),
                mybir.AxisListType.X, mybir.AluOpType.max)
            # broadcast global [imax, -imin] to every partition
            gmm = pers.tile([P, 2], dt.float32, tag="gmm")
            nc.gpsimd.partition_broadcast(gmm[:], g1[:])
            negmin = gmm[:, 1:2]
            rng_t = pers.tile([P, 1], dt.float32, tag="rng")
            nc.vector.tensor_tensor(rng_t[:], gmm[:, 0:1], negmin,
                                    mybir.AluOpType.add)
            lnr = pers.tile([P, 1], dt.float32, tag="lnr")
            nc.scalar.activation(lnr[:], rng_t[:],
                                 mybir.ActivationFunctionType.Ln)
            eb = pers.tile([P, 1], dt.float32, tag="eb")
            nc.vector.tensor_scalar(eb[:], lnr[:], -GAMMA, None,
                                    mybir.AluOpType.mult)

            # ---- CITY staircase (independent of the collective):
            #      cp = CITY[seg] - CITY[0]
            for ch in range(NCH):
                sl = slice(ch * CW, (ch + 1) * CW)
                sa = work.tile([P, CW], dt.float32, tag="csa")
                sb = work.tile([P, CW], dt.float32, tag="csb")
                cur = None
                for j in range(9):
                    k = 1 + 2 * j
                    outt = cp[:, sl] if j == 8 else (sa[:] if j % 2 == 0 else sb[:])
                    if cur is None:
                        nc.vector._custom_dve(
                            STAIR2_INIT, out=outt, in0=seg[:, sl],
                            s0=float(k), s1=float(_DC[k - 1]), imm2=float(_DC[k]))
                    else:
                        nc.vector._custom_dve(
                            STAIR2, out=outt, in0=seg[:, sl], in1=cur,
                            s0=float(k), s1=float(_DC[k - 1]), imm2=float(_DC[k]))
                    cur = outt

            # ---- phase 2: ln, exp (batched per ACT table set), w-add, CMR ----
            for ch in range(NCH):
                sl = slice(ch * CW, (ch + 1) * CW)
                nc.scalar.activation(v[:, sl], v[:, sl],
                                     mybir.ActivationFunctionType.Ln,
                                     bias=negmin, scale=1.0)
            for ch in range(NCH):
                sl = slice(ch * CW, (ch + 1) * CW)
                nc.scalar.activation(v[:, sl], v[:, sl],
                                     mybir.ActivationFunctionType.Exp,
                                     bias=eb[:, 0:1], scale=GAMMA)
            accs = [None, None, None]
            for ch in range(NCH):
                sl = slice(ch * CW, (ch + 1) * CW)
                w = work.tile([P, CW], dt.float32, tag="w")
                nc.vector.tensor_tensor(w[:], v[:, sl], cp[:, sl],
                                        mybir.AluOpType.add)
                for c in range(C):
                    rf = work.tile([P, CW], dt.float32, tag=f"rf{c}")
                    nc.sync.dma_start(rf[:], refl_d[c, :, sl])
                    gout = work.tile([P, CW], dt.float32, tag="gout")
                    accn = work.tile([P, 1], dt.float32, tag=f"acc{c}")
                    seed = 0.0 if accs[c] is None else accs[c][:]
                    nc.vector._custom_dve(
                        CLIPMULRED, out=gout[:], in0=w[:], in1=rf[:],
                        s0=_C0, s1=seed, imm2=0.0, accum_out=accn[:])
                    accs[c] = accn
            for c in range(C):
                nc.sync.dma_start(out_d[:, c:c + 1], accs[c][:])

    nc.compile()
    return nc


def _get_compiled():
    global _COMPILED
    if _COMPILED is None:
        _COMPILED = _build()
    return _COMPILED


# ---------------------------------------------------------------- entry point

def kernel(reflectance, org_img, seg_label, _trace=False):
    reflectance = np.asarray(reflectance, dtype=np.float32)
    org_img = np.asarray(org_img, dtype=np.float32)
    seg_label = np.asarray(seg_label)
    if seg_label.dtype != np.int32:
        seg_label = seg_label.astype(np.int32)

    nc = _get_compiled()
    in_maps = []
    for i in range(N_CORES):
        in_maps.append({
            "org": np.ascontiguousarray(org_img[i].reshape(C, P, F)),
            "seg": np.ascontiguousarray(seg_label[i].reshape(P, F)),
            "refl": np.ascontiguousarray(reflectance[i].reshape(C, P, F)),
        })
    res = run_bass_kernel_spmd(nc, in_maps, core_ids=list(range(N_CORES)),
                               trace=_trace)
    totals = np.zeros(C, dtype=np.float64)
    for i in range(N_CORES):
        totals += res.results[i]["out"].astype(np.float64).sum(axis=0)
    means = totals / float(B * H * W_IMG)
    r, g, b = means[0], means[1], means[2]
    loss = LOSS_WEIGHT * ((r - g) ** 2 + (r - b) ** 2 + (g - b) ** 2)
    if _trace:
        kernel._last_exec_time_ns = res.exec_time_ns
        kernel._last_results = res
    return np.float32(loss)


# revision 16
# speedup vs baseline: 1.7994x; 1.7994x over previous
"""Trainium2 Bass kernel for ColorProcessingLossV2 (8-core data-parallel).

Pipeline per core (one batch image, 3x512x1024 pixels laid out as [128, 4096]):
  phase 1: chanmax = max over RGB of org_img; v = chanmax - NIGHT[seg]
           (19-entry gather realized as an 18-step staircase of custom
           fused DVE ops, 2 steps per op); local min/max of v.
  sync:    AllGather of [vmax, -vmin] across the 8 cores; each core
           reduces to the global imin/imax.
  phase 2: g = ((v - imin)/(imax - imin))^0.4 computed as
           exp(0.4*ln(v - imin) - 0.4*ln(imax - imin)) on the ACT engine;
           w = g + (CITY[seg] - CITY[0]) via the second staircase;
           per-channel partial sums of min(w + CITY[0], 1) * reflectance
           via a fused clip-mul-reduce DVE op.
  output:  per-core [128, 3] per-partition channel sums; host reduces in
           float64 and forms the scalar loss.
"""

import os
import sys

for _p in ("/root/.axon_site/_ro/trn_rl_repo", "/opt/trn_rl_repo"):
    if _p not in sys.path:
        sys.path.append(_p)

import numpy as np


def _install_ntff_hook_shim():
    """Provide antenv.axon_hooks (NTFF profiling hook) when the image lacks it."""
    try:
        import antenv.axon_hooks  # noqa: F401
        return
    except ImportError:
        pass
    import contextlib
    import ctypes
    import types

    so_path = "/opt/axon/libaxon_pjrt.so"
    state = {"hook": None}

    def _make_hook():
        if not os.path.exists(so_path):
            return None
        lib = ctypes.CDLL(so_path)
        if not hasattr(lib, "axon_start_nrt_profile"):
            return None
        lib.axon_start_nrt_profile.argtypes = [
            ctypes.POINTER(ctypes.c_int64), ctypes.c_size_t]
        lib.axon_start_nrt_profile.restype = ctypes.c_int64
        lib.axon_stop_nrt_profile.argtypes = [ctypes.c_char_p]
        lib.axon_stop_nrt_profile.restype = ctypes.c_int64

        @contextlib.contextmanager
        def _hook(output_dir, device_ids):
            import jax
            jax.devices()
            if device_ids:
                ids = (ctypes.c_int64 * len(device_ids))(*device_ids)
                rc = lib.axon_start_nrt_profile(ids, len(device_ids))
            else:
                rc = lib.axon_start_nrt_profile(None, 0)
            if rc != 0:
                raise RuntimeError(f"axon_start_nrt_profile rc={rc}")
            try:
                yield
            finally:
                n = lib.axon_stop_nrt_profile(str(output_dir).encode())
                print(f"profile: {n} file(s) written to {output_dir}",
                      file=sys.stderr)

        return _hook

    mod = types.ModuleType("antenv.axon_hooks")

    def get_axon_ntff_profile_hook():
        if state["hook"] is None:
            state["hook"] = _make_hook()
        return state["hook"]

    def set_axon_ntff_profile_hook(h):
        state["hook"] = h

    mod.get_axon_ntff_profile_hook = get_axon_ntff_profile_hook
    mod.set_axon_ntff_profile_hook = set_axon_ntff_profile_hook
    sys.modules["antenv.axon_hooks"] = mod


_install_ntff_hook_shim()

import concourse.bacc as bacc
import concourse.bass_isa as bass_isa
import concourse.mybir as mybir
import concourse.tile as tile
import concourse.dve_ops as dve_ops
from concourse.bass_utils import run_bass_kernel_spmd
from concourse.dve_spec import Spec, Src0, Src1, C0, C1, C2, One, minn, maxx, lower, _has_src1
from concourse.dve_uop import DveOpSpec
from operator import add as _operator_add

# ---------------------------------------------------------------- constants

_NIGHT = np.array([
    76.5113984140019, 76.23163212875781, 60.90662084364415, 69.06930071129905,
    69.63671393061327, 73.11413822794262, 140.7827781957324, 116.29554873008291,
    46.23329954488532, 57.839322341112386, 32.61465346757989, 57.4385179294615,
    62.234896087294814, 90.90285758569436, 91.99610158117673, 91.82209397173472,
    94.06478985576457, 74.6924145472464, 69.15034088822232], dtype=np.float64) / 255.0
_CITY = np.array([
    86.46051320057052, 79.37014543897092, 95.30679177391578, 71.11888521745776,
    75.57026559270716, 77.90493757655786, 74.77466800282637, 88.27701037425895,
    57.685269557270146, 72.71472387765841, 229.9589238353863, 66.9194012998903,
    60.42471796718752, 76.8407421534007, 74.98657626719087, 73.56771430328095,
    123.92515568872523, 68.93476495876828, 76.0970460111028], dtype=np.float64) / 255.0

# staircase deltas over the fp32-rounded tables (matches the reference's
# fp32 table values at every integer label)
_DN = np.diff(_NIGHT.astype(np.float32).astype(np.float64))
_DC = np.diff(_CITY.astype(np.float32).astype(np.float64))
_C0 = float(np.float32(_CITY[0]))

GAMMA = 0.4
LOSS_WEIGHT = 1.0

B, C, H, W_IMG = 8, 3, 512, 1024
P = 128
F = (H * W_IMG) // P          # 4096 free elements per partition
N_CORES = 8
NCH = 4                        # chunks per phase
CW = F // NCH                  # chunk width

# ---------------------------------------------------------------- custom ops


def _register_dve_op(name, spec, subdim=False):
    for op in dve_ops.OPS:
        if op.name == name:
            return op
    row = dve_ops._CUSTOM_DVE_ROW_BASE + len(dve_ops.OPS)
    assert row < 0x20, "custom DVE opcode rows exhausted"
    shas = {}
    for ver in ("v3", "v4"):
        tmp = DveOpSpec(name=name, opcode=row, uops=lower(spec, ver=ver),
                        rd1_en=_has_src1(spec))
        shas[ver] = tmp.sha(ver)
    op = dve_ops.DveOp(name, spec, subdim=subdim, uops_sha=shas)
    dve_ops.OPS.append(op)
    dve_ops.CUSTOM_DVE_SPECS[name] = spec
    dve_ops._SUB_OPCODE_FOR_NAME[name] = row
    return op


def _stair2_ref(in0, in1, s0, s1, imm2):
    x = in0.astype(np.float32)
    return (in1 + (x >= s0) * np.float32(s1) + (x > s0) * np.float32(imm2)
            ).astype(np.float32)


# out = acc + (seg >= k) * d0 + (seg > k) * d1   (two staircase steps)
STAIR2 = _register_dve_op(
    "STAIR2_ANT",
    Spec(body=Src1 + (Src0 >= C0) * C1 + (Src0 > C0) * C2,
         reference=_stair2_ref),
)


def _stair2_amax_ref(in0, in1, s0, s1, imm2):
    body = _stair2_ref(in0, in1, s0, s1, imm2)
    acc = np.maximum.reduce(body.reshape(body.shape[0], -1), axis=-1,
                            keepdims=True)
    return body, acc.astype(np.float32)


# STAIR2 + running max of the output (used on the last NIGHT step)
STAIR2_AMAX = _register_dve_op(
    "STAIR2_AMAX_ANT",
    Spec(body=Src1 + (Src0 >= C0) * C1 + (Src0 > C0) * C2,
         accum=maxx, reference=_stair2_amax_ref),
)


def _stair2_init_ref(in0, in1, s0, s1, imm2):
    x = in0.astype(np.float32)
    return ((x >= s0) * np.float32(s1) + (x > s0) * np.float32(imm2)
            ).astype(np.float32)


# first staircase op of a zero-seeded chain (no accumulator input)
STAIR2_INIT = _register_dve_op(
    "STAIR2_INIT_ANT",
    Spec(body=(Src0 >= C0) * C1 + (Src0 > C0) * C2,
         reference=_stair2_init_ref),
)


def _cmr_ref(in0, in1, s0, s1, imm2):
    body = (np.minimum(in0.astype(np.float32) + np.float32(s0), np.float32(1.0))
            * in1).astype(np.float32)
    acc = (np.asarray(s1, np.float32).reshape(-1, 1)
           + body.reshape(body.shape[0], -1).sum(axis=-1, keepdims=True))
    return body, acc.astype(np.float32)


# out = min(w + c0, 1) * refl ; accum_out = seed + sum(out)
CLIPMULRED = _register_dve_op(
    "CLIPMULRED_ANT",
    Spec(body=minn(Src0 + C0, One) * Src1, accum=_operator_add,
         accum_init=C1, reference=_cmr_ref),
)

# ---------------------------------------------------------------- bass build

_COMPILED = None


def _build():
    dt = mybir.dt
    nc = bacc.Bacc("TRN2", target_bir_lowering=False, debug=False,
                   num_devices=N_CORES)
    org_d = nc.dram_tensor("org", [C, P, F], dt.float32, kind="ExternalInput").ap()
    seg_d = nc.dram_tensor("seg", [P, F], dt.int32, kind="ExternalInput").ap()
    refl_d = nc.dram_tensor("refl", [C, P, F], dt.float32, kind="ExternalInput").ap()
    out_d = nc.dram_tensor("out", [P, C], dt.float32, kind="ExternalOutput").ap()

    with tile.TileContext(nc) as tc:
        with (
            tc.tile_pool(name="pers", bufs=1) as pers,
            tc.tile_pool(name="work", bufs=2) as work,
            tc.tile_pool(name="dram", bufs=1, space="DRAM") as dram,
        ):
            seg = pers.tile([P, F], dt.int32, tag="seg")
            v = pers.tile([P, F], dt.float32, tag="v")
            cp = pers.tile([P, F], dt.float32, tag="cp")

            # ---- phase 1: chanmax, NIGHT staircase, local min/max ----
            mxs, mns = [], []
            for ch in range(NCH):
                sl = slice(ch * CW, (ch + 1) * CW)
                nc.sync.dma_start(seg[:, sl], seg_d[:, sl])
                o01 = work.tile([P, 2 * CW], dt.float32, tag="org01")
                nc.sync.dma_start(o01[:, 0:CW], org_d[0, :, sl])
                nc.sync.dma_start(o01[:, CW:2 * CW], org_d[1, :, sl])
                o2 = work.tile([P, CW], dt.float32, tag="org2")
                nc.sync.dma_start(o2[:], org_d[2, :, sl])
                cmx = work.tile([P, CW], dt.float32, tag="cmx")
                nc.vector.tensor_tensor(cmx[:], o01[:, 0:CW], o01[:, CW:2 * CW],
                                        mybir.AluOpType.max)
                nc.vector.tensor_tensor(cmx[:], cmx[:], o2[:],
                                        mybir.AluOpType.max)
                # NIGHT staircase: v = chanmax - (NIGHT[seg] - NIGHT[0])
                sa = work.tile([P, CW], dt.float32, tag="sa")
                cur = cmx[:]
                mx = pers.tile([P, 1], dt.float32, tag=f"mx{ch}")
                for j in range(9):
                    k = 1 + 2 * j
                    if j == 8:
                        nc.vector._custom_dve(
                            STAIR2_AMAX, out=v[:, sl], in0=seg[:, sl], in1=cur,
                            s0=float(k), s1=float(-_DN[k - 1]),
                            imm2=float(-_DN[k]), accum_out=mx[:])
                    else:
                        outt = sa[:] if j % 2 == 0 else cmx[:]
                        nc.vector._custom_dve(
                            STAIR2, out=outt, in0=seg[:, sl], in1=cur,
                            s0=float(k), s1=float(-_DN[k - 1]),
                            imm2=float(-_DN[k]))
                        cur = outt
                mn = pers.tile([P, 1], dt.float32, tag=f"mn{ch}")
                nc.vector.tensor_reduce(mn[:], v[:, sl], mybir.AxisListType.X,
                                        mybir.AluOpType.min)
                mxs.append(mx)
                mns.append(mn)

            # combine chunk extremes -> [128, 2] = [vmax, -vmin]
            mm = pers.tile([P, 2], dt.float32, tag="mm")
            nc.vector.tensor_tensor(mm[:, 0:1], mxs[0][:], mxs[1][:],
                                    mybir.AluOpType.max)
            for t in mxs[2:]:
                nc.vector.tensor_tensor(mm[:, 0:1], mm[:, 0:1], t[:],
                                        mybir.AluOpType.max)
            nc.vector.tensor_tensor(mm[:, 1:2], mns[0][:], mns[1][:],
                                    mybir.AluOpType.min)
            for t in mns[2:]:
                nc.vector.tensor_tensor(mm[:, 1:2], mm[:, 1:2], t[:],
                                        mybir.AluOpType.min)
            nc.vector.tensor_scalar(mm[:, 1:2], mm[:, 1:2], -1.0, None,
                                    mybir.AluOpType.mult)

            # gather the 128 per-partition pairs onto partition 0 via DMA,
            # reduce there, AllGather the per-core [vmax, -vmin]
            flat = pers.tile([1, 2 * P], dt.float32, tag="flat")
            nc.sync.dma_start(flat[:], mm[:])
            l1 = pers.tile([1, 2], dt.float32, tag="l1")
            nc.vector.tensor_reduce(
                l1[:], flat[:].rearrange("p (c two) -> p two c", two=2),
                mybir.AxisListType.X, mybir.AluOpType.max)
            agi = dram.tile([1, 2], dt.float32)
            ago = dram.tile([N_CORES, 2], dt.float32, addr_space="Shared")
            nc.sync.dma_start(agi[:], l1[:])
            nc.gpsimd.collective_compute(
                "AllGather", mybir.AluOpType.bypass,
                replica_groups=[list(range(N_CORES))],
                ins=[agi.opt()], outs=[ago.opt()])
            agt = pers.tile([1, 2 * N_CORES], dt.float32, tag="agt")
            nc.sync.dma_start(agt[:], ago[:])
            g1 = pers.tile([1, 2], dt.float32, tag="g1")
            nc.vector.tensor_reduce(
                g1[:], agt[:].rearrange("p (c two) -> p two c", two=2),
                mybir.AxisListType.X, mybir.AluOpType.max)
            # broadcast global [imax, -imin] to every partition
            gmm = pers.tile([P, 2], dt.float32, tag="gmm")
            nc.gpsimd.partition_broadcast(gmm[:], g1[:])
            negmin = gmm[:, 1:2]
            rng_t = pers.tile([P, 1], dt.float32, tag="rng")
            nc.vector.tensor_tensor(rng_t[:], gmm[:, 0:1], negmin,
                                    mybir.AluOpType.add)
            lnr = pers.tile([P, 1], dt.float32, tag="lnr")
            nc.scalar.activation(lnr[:], rng_t[:],
                                 mybir.ActivationFunctionType.Ln)
            eb = pers.tile([P, 1], dt.float32, tag="eb")
            nc.vector.tensor_scalar(eb[:], lnr[:], -GAMMA, None,
                                    mybir.AluOpType.mult)

            # ---- CITY staircase (independent of the collective):
            #      cp = CITY[seg] - CITY[0]
            for ch in range(NCH):
                sl = slice(ch * CW, (ch + 1) * CW)
                sa = work.tile([P, CW], dt.float32, tag="csa")
                sb = work.tile([P, CW], dt.float32, tag="csb")
                cur = None
                for j in range(9):
                    k = 1 + 2 * j
                    outt = cp[:, sl] if j == 8 else (sa[:] if j % 2 == 0 else sb[:])
                    if cur is None:
                        nc.vector._custom_dve(
                            STAIR2_INIT, out=outt, in0=seg[:, sl],
                            s0=float(k), s1=float(_DC[k - 1]), imm2=float(_DC[k]))
                    else:
                        nc.vector._custom_dve(
                            STAIR2, out=outt, in0=seg[:, sl], in1=cur,
                            s0=float(k), s1=float(_DC[k - 1]), imm2=float(_DC[k]))
                    cur = outt

            # ---- phase 2: ln, exp (batched per ACT table set), w-add, CMR ----
            for ch in range(NCH):
                sl = slice(ch * CW, (ch + 1) * CW)
                nc.scalar.activation(v[:, sl], v[:, sl],
                                     mybir.ActivationFunctionType.Ln,
                                     bias=negmin, scale=1.0)
            for ch in range(NCH):
                sl = slice(ch * CW, (ch + 1) * CW)
                nc.scalar.activation(v[:, sl], v[:, sl],
                                     mybir.ActivationFunctionType.Exp,
                                     bias=eb[:, 0:1], scale=GAMMA)
            accs = [None, None, None]
            for ch in range(NCH):
                sl = slice(ch * CW, (ch + 1) * CW)
                w = work.tile([P, CW], dt.float32, tag="w")
                nc.vector.tensor_tensor(w[:], v[:, sl], cp[:, sl],
                                        mybir.AluOpType.add)
                for c in range(C):
                    rf = work.tile([P, CW], dt.float32, tag=f"rf{c}")
                    nc.sync.dma_start(rf[:], refl_d[c, :, sl])
                    gout = work.tile([P, CW], dt.float32, tag="gout")
                    accn = work.tile([P, 1], dt.float32, tag=f"acc{c}")
                    seed = 0.0 if accs[c] is None else accs[c][:]
                    nc.vector._custom_dve(
                        CLIPMULRED, out=gout[:], in0=w[:], in1=rf[:],
                        s0=_C0, s1=seed, imm2=0.0, accum_out=accn[:])
                    accs[c] = accn
            for c in range(C):
                nc.sync.dma_start(out_d[:, c:c + 1], accs[c][:])

    nc.compile()
    return nc


def _get_compiled():
    global _COMPILED
    if _COMPILED is None:
        _COMPILED = _build()
    return _COMPILED


# ---------------------------------------------------------------- entry point

def kernel(reflectance, org_img, seg_label, _trace=False):
    reflectance = np.asarray(reflectance, dtype=np.float32)
    org_img = np.asarray(org_img, dtype=np.float32)
    seg_label = np.asarray(seg_label)
    if seg_label.dtype != np.int32:
        seg_label = seg_label.astype(np.int32)

    nc = _get_compiled()
    in_maps = []
    for i in range(N_CORES):
        in_maps.append({
            "org": np.ascontiguousarray(org_img[i].reshape(C, P, F)),
            "seg": np.ascontiguousarray(seg_label[i].reshape(P, F)),
            "refl": np.ascontiguousarray(reflectance[i].reshape(C, P, F)),
        })
    res = run_bass_kernel_spmd(nc, in_maps, core_ids=list(range(N_CORES)),
                               trace=_trace)
    totals = np.zeros(C, dtype=np.float64)
    for i in range(N_CORES):
        totals += res.results[i]["out"].astype(np.float64).sum(axis=0)
    means = totals / float(B * H * W_IMG)
    r, g, b = means[0], means[1], means[2]
    loss = LOSS_WEIGHT * ((r - g) ** 2 + (r - b) ** 2 + (g - b) ** 2)
    if _trace:
        kernel._last_exec_time_ns = res.exec_time_ns
        kernel._last_results = res
    return np.float32(loss)
